# revision 1
# baseline (speedup 1.0000x reference)
"""Trainium2 Bass kernel v2 for nn_Model1_52518860096440.

Reference (B=4, S=4096, HID=1024, H=16, DH=64):
    qkv = query @ W_qkv.T + b_qkv          # only `query` used
    q,k,v = split(qkv) -> (B,S,H,DH)
    s[t,h,g] = q[t,h]·k[t,g]/8 + mask[t,h,g]
    p = softmax_g(s);  o[t,h] = sum_g p[t,h,g] v[t,g]

Strategy (per core, 2048 tokens, 4 chunks of 512):
  - qT/kT computed transposed: PSUM (channel-block, token) via lhsT=W^T
    chunks, rhs=xT.  ACT copies to SBUF f16 with per-partition bias.
  - score products on DVE/GPSIMD in (d, token) layout, 2 head-pairs per
    mul (parity-aligned via a partition-swapped kT copy).
  - score reduction over d via PE ones-mask matmuls (2 cols out) ->
    scores land token-partitioned in PSUM.
  - softmax on DVE/ACT; p pre-normalized; p scattered into persistent
    zeroed block-diagonal lhsT tiles (8 per-u DMAs).
  - v computed token-layout, bounced through DRAM to (token%8,g)-grouped
    layout; AV = 16 block-diag matmuls + bias matmuls per tile.
  - output stored grouped; host unpermutes.
"""

from contextlib import ExitStack

import numpy as np

B, S, HID, H = 4, 4096, 1024, 16
DH = HID // H
NCORES = 8
T = B * S
TC = T // NCORES              # 2048 tokens/core
P = 128
NT = TC // P                  # 16 tiles/core
CH = 512                      # tokens per chunk
NCH = TC // CH                # 4 chunks
TPC = CH // P                 # 4 tiles per chunk
GPS_MUL_MOD = 6               # every 6th score mul on GPSIMD

_compiled = {}


def _cap(ap, dims, offset=None):
    """Copy `ap`, replace dims; `offset` is ADDED to the existing offset."""
    a = ap.copy()
    a.ap.clear()
    a.ap.extend([tuple(d) for d in dims])
    if offset is not None:
        a.offset = a.offset + offset
    return a


def _build(phase=4):
    import concourse.bass as bass
    import concourse.tile as tile
    import concourse.mybir as mybir
    from concourse import bacc

    f32 = mybir.dt.float32
    f16 = mybir.dt.float16
    Alu = mybir.AluOpType
    Act = mybir.ActivationFunctionType

    nc = bacc.Bacc("TRN2", target_bir_lowering=False, debug=False,
                   num_devices=NCORES)

    xT_d = nc.dram_tensor("xT", (HID, TC), f16, kind="ExternalInput")
    wqk_d = nc.dram_tensor("wqk", (HID, 2 * HID), f16, kind="ExternalInput")
    wv_d = nc.dram_tensor("wv", (HID, HID), f16, kind="ExternalInput")
    bqk_d = nc.dram_tensor("bqk", (P, 16), f32, kind="ExternalInput")
    bvg_d = nc.dram_tensor("bvg", (P, DH), f16, kind="ExternalInput")
    mask_d = nc.dram_tensor("maskp", (TC, H * H), f16, kind="ExternalInput")
    vstg_d = nc.dram_tensor("vstg", (NT, P, HID), f16, kind="Internal")
    pstg_d = nc.dram_tensor("pstg", (NT, P, H * H), f16, kind="Internal")
    out_d = nc.dram_tensor("out", (NT, P, HID), f32, kind="ExternalOutput")

    with tile.TileContext(nc) as tc, ExitStack() as ctx:
        const = ctx.enter_context(tc.tile_pool(name="const", bufs=1))
        xtp = ctx.enter_context(tc.tile_pool(name="xt", bufs=2))
        qkp = ctx.enter_context(tc.tile_pool(name="qk", bufs=2))
        prodp = ctx.enter_context(tc.tile_pool(name="prod", bufs=8))
        vgp = ctx.enter_context(tc.tile_pool(name="vg", bufs=3))
        vtkp = ctx.enter_context(tc.tile_pool(name="vtk", bufs=3))
        smp = ctx.enter_context(tc.tile_pool(name="sm", bufs=4))
        avp = ctx.enter_context(tc.tile_pool(name="av", bufs=2))
        mskp = ctx.enter_context(tc.tile_pool(name="msk", bufs=2))
        pq = ctx.enter_context(tc.tile_pool(name="pq", bufs=2, space="PSUM"))
        pv = ctx.enter_context(tc.tile_pool(name="pv", bufs=2, space="PSUM"))
        pss = ctx.enter_context(tc.tile_pool(name="pss", bufs=1, space="PSUM"))
        pav = ctx.enter_context(tc.tile_pool(name="pav", bufs=1, space="PSUM"))

        # ---------- resident constants ----------
        # wqk_sb[p, kb*2048 + cb*128 + c] = wqk[kb*128+p, cb*128+c]
        # loaded k-half first (kT blocks are consumed first)
        wqk_sb = const.tile([P, 16 * HID], f16, tag="wqk")
        nc.sync.dma_start(
            _cap(wqk_sb[:], [[16 * HID, P], [2 * HID, 8], [1, HID]],
                 offset=HID),
            _cap(wqk_d[:], [[2 * HID, P], [P * 2 * HID, 8], [1, HID]],
                 offset=HID))
        # wv_sb[p, kb*1024 + c] = wv[kb*128+p, c]
        wv_sb = const.tile([P, 8 * HID], f16, tag="wv")

        def deferred_w_loads():
            # q-half (used from slot 8) and wv (used after slot 15) load
            # behind round 0's xt/mask so the first matmuls start sooner
            nc.sync.dma_start(
                _cap(wqk_sb[:], [[16 * HID, P], [2 * HID, 8], [1, HID]]),
                _cap(wqk_d[:], [[2 * HID, P], [P * 2 * HID, 8], [1, HID]]))
            nc.sync.dma_start(
                _cap(wv_sb[:], [[8 * HID, P], [HID, 8], [1, HID]]),
                _cap(wv_d[:], [[HID, P], [P * HID, 8], [1, HID]]))
        bqk_sb = const.tile([P, 16], f32, tag="bqk")
        nc.sync.dma_start(bqk_sb[:], bqk_d[:])
        bvg_sb = const.tile([P, DH], f16, tag="bvg")
        nc.sync.dma_start(bvg_sb[:], bvg_d[:])

        neg4 = const.tile([P, 1], f32, tag="neg4")
        nc.vector.memset(neg4[:], -4.0)
        zeros512 = const.tile([P, CH], f16, tag="zeros512")
        nc.vector.memset(zeros512[:], 0.0)
        ones2 = const.tile([P, 2], f16, tag="ones2")   # [upper, lower]
        nc.vector.memset(ones2[0:64, 0:1], 1.0)
        nc.vector.memset(ones2[0:64, 1:2], 0.0)
        nc.vector.memset(ones2[64:128, 0:1], 0.0)
        nc.vector.memset(ones2[64:128, 1:2], 1.0)

        # persistent block-diagonal lhsT tiles (zeroed once; scatters only
        # ever write the diagonal blocks)
        Lbufs = []
        for i in range(2):
            Lt = const.tile([P, 16 * P], f16, tag=f"L{i}")
            nc.vector.memset(Lt[:], 0.0)
            Lbufs.append(Lt)

        # ---------- interleaved emission ----------
        state = {}
        state_vgs = {}

        def emit_round(c_a, b_spec):
            # b_spec: None or (chunk, half) with half in (None, 0, 1)
            if c_a is not None:
                xt = xtp.tile([P, 8 * CH], f16, tag="xt")
                nc.sync.dma_start(
                    _cap(xt[:], [[8 * CH, P], [CH, 8], [1, CH]]),
                    _cap(xT_d[:], [[TC, P], [P * TC, 8], [1, CH]],
                         offset=c_a * CH))
                qT = qkp.tile([P, 8 * CH], f16, tag="qT")
                kT = qkp.tile([P, 8 * CH], f16, tag="kT")
                kTs = qkp.tile([P, 8 * CH], f16, tag="kTs")
                msk = mskp.tile([P, TPC * 256], f16, tag="msk")
                nc.sync.dma_start(
                    _cap(msk[:], [[TPC * 256, P], [256, TPC], [1, 256]]),
                    _cap(mask_d[:], [[256, P], [P * 256, TPC], [1, 256]],
                         offset=c_a * CH * 256))

            if c_a == 0:
                deferred_w_loads()
            c_b = half = None
            if b_spec is not None:
                c_b, half = b_spec
                qTb, kTb, kTsb, mskb = state[c_b]
                col0 = 0 if half is None else half * 256
                wcols = 512 if half is None else 256
                tiles = list(range(TPC)) if half is None \
                    else [2 * half, 2 * half + 1]
                ns2 = len(tiles) // 2
                s2 = [pss.tile([P, 512], f32, tag=f"s{i}", name=f"s_ps{i}")
                      for i in range(ns2)]

                def s_ap(t, c0, n):
                    ti = tiles.index(t)
                    return s2[ti // 2][:, (ti % 2) * 256 + c0:
                                       (ti % 2) * 256 + c0 + n]

                combos = [(ib, sw, jh) for ib in range(8)
                          for sw in range(2) for jh in range(2)]

            # --- 16 slots: A qkT block + B mul-units ---
            for slot in range(16):
                if c_a is not None:
                    cb = (slot + 8) % 16   # kT blocks first
                    acc = pq.tile([P, CH], f32, tag="qkacc")
                    for kb in range(8):
                        nc.tensor.matmul(
                            acc[:],
                            wqk_sb[:, kb * 2048 + cb * P:
                                   kb * 2048 + (cb + 1) * P],
                            xt[:, kb * CH:(kb + 1) * CH],
                            start=(kb == 0), stop=(kb == 7))
                    blk = qT if cb < 8 else kT
                    col = (cb % 8) * CH
                    nc.scalar.activation(blk[:, col:col + CH], acc[:],
                                         Act.Identity,
                                         bias=bqk_sb[:, cb:cb + 1], scale=1.0)
                    if slot == 7:
                        nc.sync.dma_start(kTs[0:64, :], kT[64:128, :])
                        nc.sync.dma_start(kTs[64:128, :], kT[0:64, :])
                if c_b is not None:
                    for q2 in range(2):
                        ui = slot * 2 + q2
                        ib, sw, jh = combos[ui]
                        # one mul covers 4 jb blocks (stride-0 qT repeat)
                        prod = prodp.tile([P, 4 * CH], f16, tag="prod")
                        kblk = kTsb if sw else kTb
                        gmod = 5 if c_a is not None else 4
                        eng = (nc.gpsimd if ui % gmod == gmod - 1
                               else nc.vector)
                        in0 = _cap(qTb[:],
                                   [[8 * CH, P], [0, 4], [1, wcols]],
                                   offset=ib * CH + col0)
                        in1 = _cap(kblk[:],
                                   [[8 * CH, P], [CH, 4], [1, wcols]],
                                   offset=jh * 4 * CH + col0)
                        oap = _cap(prod[:],
                                   [[4 * CH, P], [CH, 4], [1, wcols]])
                        eng.tensor_tensor(oap, in0, in1,
                                          op=Alu.mult)
                        for jj in range(4):
                            jb = jh * 4 + jj
                            cpk = 2 * (8 * jb + ib) + (128 if sw else 0)
                            lastu = (ib == 7 and sw == 1 and jh == 1
                                     and jj == 3)
                            for t in tiles:
                                tix = tiles.index(t)
                                lo = jj * CH + (t - tiles[0]) * P
                                nc.tensor.matmul(
                                    s_ap(t, cpk, 2),
                                    prod[:, lo:lo + P],
                                    ones2[:],
                                    start=(ui == 0 and jj == 0
                                           and tix % 2 == 0),
                                    stop=(lastu and tix % 2 == 1))

            if c_b is not None and phase <= 1:
                for t in tiles:
                    sc = smp.tile([P, 256], f32, tag="sdbg")
                    nc.vector.tensor_copy(sc[:], s_ap(t, 0, 256))
                    nc.sync.dma_start(out_d[c_b * TPC + t][:, 0:256], sc[:])
                c_b = None

            # --- per-tile: A v-matmuls + B softmax/AV ---
            vgs = [] if c_a is not None else None
            nt_seg = max(TPC if c_a is not None else 0,
                         len(tiles) if c_b is not None else 0)
            for ti in range(nt_seg):
                if c_a is not None and ti < TPC:
                    t = ti
                    vtk = vtkp.tile([P, HID], f16, tag="vtk")
                    for oc in range(2):
                        acc = pv.tile([P, CH], f32, tag="vacc")
                        for kb in range(8):
                            nc.tensor.matmul(
                                acc[:],
                                xt[:, kb * CH + t * P: kb * CH + (t + 1) * P],
                                wv_sb[:, kb * HID + oc * CH:
                                      kb * HID + (oc + 1) * CH],
                                start=(kb == 0), stop=(kb == 7))
                        nc.scalar.copy(vtk[:, oc * CH:(oc + 1) * CH], acc[:])
                    gt_a = c_a * TPC + t
                    nc.scalar.dma_start(vstg_d[gt_a], vtk[:])
                    vg = vgp.tile([P, HID], f16, tag="vg")
                    nc.scalar.dma_start(
                        _cap(vg[:], [[HID, P], [DH, 16], [1, DH]]),
                        _cap(vstg_d[gt_a], [[DH, P], [8 * HID, 16], [1, DH]]))
                    vgs.append(vg)

                if c_b is None or ti >= len(tiles):
                    continue
                t = tiles[ti]
                # ---- B: softmax for tile t of chunk c_b ----
                gt = c_b * TPC + t
                sm = smp.tile([P, 256], f16, tag="sm")
                for hp in range(2):
                    for bb in range(2):
                        gp = hp ^ bb
                        tix = tiles.index(t)
                        in0 = _cap(s2[tix // 2][:],
                                   [[512, P], [16, 8], [2, 8]],
                                   offset=(tix % 2) * 256 + bb * 128 + hp)
                        in1 = _cap(mskb[:],
                                   [[TPC * 256, P], [32, 8], [2, 8]],
                                   offset=t * 256 + 16 * gp + hp)
                        oap = _cap(sm[:], [[256, P], [32, 8], [2, 8]],
                                   offset=16 * gp + hp)
                        nc.vector.tensor_add(oap, in0, in1)
                e = smp.tile([P, 256], f16, tag="e")
                nc.scalar.activation(e[:], sm[:], Act.Exp, bias=neg4[:])
                sums = smp.tile([P, 16], f32, tag="sums")
                nc.vector.tensor_reduce(
                    sums[:], e[:].rearrange("p (g h) -> p h g", g=16),
                    axis=mybir.AxisListType.X, op=Alu.add)
                recip = smp.tile([P, 16], f16, tag="recip")
                with nc.allow_low_precision(reason="softmax recip f16 ok"):
                    nc.vector.reciprocal(recip[:], sums[:])
                p_t = smp.tile([P, 256], f16, tag="p")
                r_b = recip[:].unsqueeze(1).broadcast_to((P, 16, 16))
                nc.vector.tensor_mul(
                    p_t[:].rearrange("p (g h) -> p g h", g=16),
                    e[:].rearrange("p (g h) -> p g h", g=16), r_b)
                if phase == 2:
                    dbg = smp.tile([P, 256], f32, tag="dbg2")
                    nc.vector.tensor_copy(dbg[:], sm[:])
                    nc.sync.dma_start(out_d[gt][:, 0:256], dbg[:])
                    continue
                if phase == 3:
                    dbg = smp.tile([P, 256], f32, tag="dbg3")
                    nc.vector.tensor_copy(dbg[:], p_t[:])
                    nc.sync.dma_start(out_d[gt][:, 0:256], dbg[:])
                    continue

                # ---- p scatter via DRAM bounce ----
                L = Lbufs[gt % 2]
                nc.sync.dma_start(pstg_d[gt], p_t[:])
                for u in range(8):
                    src = _cap(pstg_d[gt],
                               [[16, 16], [8 * 256, 16], [1, 16]],
                               offset=u * 256)
                    dst = _cap(L[:], [[16 * P, 16], [P, 16], [1, 16]],
                               offset=u * (16 * 16 * P + 16))
                    gmod_r = 2 if c_a is None else 4
                    eng = (nc.gpsimd if u % gmod_r == gmod_r - 1
                           else nc.sync)
                    eng.dma_start(dst, src)

                # ---- AV matmuls ----
                vgb = state_vgs[c_b][t]
                ps_a = pav.tile([P, CH], f32, tag="av0")
                ps_b = pav.tile([P, CH], f32, tag="av1")
                for j in range(16):
                    tgt = ps_a if j < 8 else ps_b
                    colo = (j % 8) * DH
                    nc.tensor.matmul(tgt[:, colo:colo + DH],
                                     L[:, j * P:(j + 1) * P],
                                     vgb[:, j * DH:(j + 1) * DH],
                                     start=(j % 8 == 0), stop=False)
                    nc.tensor.matmul(tgt[:, colo:colo + DH],
                                     L[:, j * P:(j + 1) * P],
                                     bvg_sb[:],
                                     start=False, stop=(j % 8 == 7))
                av = avp.tile([P, HID], f32, tag="avsb")
                nc.scalar.copy(av[:, 0:CH], ps_a[:])
                nc.scalar.copy(av[:, CH:HID], ps_b[:])
                nc.scalar.dma_start(out_d[gt], av[:])

            if c_b is not None and (half is None or half == 1):
                state.pop(c_b, None)
                state_vgs.pop(c_b, None)
            if c_a is not None:
                state[c_a] = (qT, kT, kTs, msk)
                state_vgs[c_a] = vgs

        if phase < 4:
            rounds = [(c if c < NCH else None,
                       (c - 1, None) if c > 0 else None)
                      for c in range(NCH + 1)]
        else:
            rounds = []
            for c in range(NCH):
                rounds.append((c, (c - 1, None) if 1 <= c < NCH else None))
            rounds.append((None, (NCH - 1, None)))
        for c_a, b_spec in rounds:
            emit_round(c_a, b_spec)

    nc.compile()
    return nc


def _host_prep(query, W_qkv, b_qkv, attn_mask):
    scale = 1.0 / np.sqrt(DH)
    x = np.ascontiguousarray(query.reshape(T, HID), dtype=np.float32)
    xT = np.ascontiguousarray(x.T).astype(np.float16)      # (HID, T)
    wT = np.array(W_qkv, dtype=np.float32).T.copy()        # (HID, 3H)
    b = np.array(b_qkv, dtype=np.float32).copy()
    wT[:, 0:HID] *= scale
    b[0:HID] *= scale
    wqk = np.ascontiguousarray(wT[:, 0:2 * HID]).astype(np.float16)
    wv = np.ascontiguousarray(wT[:, 2 * HID:]).astype(np.float16)
    bqk = np.ascontiguousarray(
        b[0:2 * HID].reshape(16, P).T).astype(np.float32)  # (128,16)
    bvg = np.ascontiguousarray(
        np.tile(b[2 * HID:].reshape(H, DH), (8, 1))).astype(np.float16)
    m = np.asarray(attn_mask, dtype=np.float32).reshape(T, H, H)
    maskp = np.ascontiguousarray(
        m.transpose(0, 2, 1).reshape(T, H * H)).astype(np.float16)
    return xT, wqk, wv, bqk, bvg, maskp


def _unpermute(res):
    # res: (NT, 128, 1024) with [tile, 16u+h, 64j+d] -> (TC, HID)
    r = res.reshape(NT, 8, H, H, DH).transpose(0, 3, 1, 2, 4)
    return np.ascontiguousarray(r).reshape(TC, HID)


def kernel(query, key, value, attn_mask, W_qkv, b_qkv):
    from concourse.bass_utils import run_bass_kernel_spmd

    xT, wqk, wv, bqk, bvg, maskp = _host_prep(query, W_qkv, b_qkv, attn_mask)

    if "nc" not in _compiled:
        _compiled["nc"] = _build()
    nc = _compiled["nc"]

    in_maps = []
    for c in range(NCORES):
        tsl = slice(c * TC, (c + 1) * TC)
        in_maps.append({
            "xT": np.ascontiguousarray(xT[:, tsl]),
            "wqk": wqk,
            "wv": wv,
            "bqk": bqk,
            "bvg": bvg,
            "maskp": np.ascontiguousarray(maskp[tsl, :]),
        })

    res = run_bass_kernel_spmd(nc, in_maps, core_ids=list(range(NCORES)))
    out = np.concatenate([_unpermute(r["out"]) for r in res.results], axis=0)
    return out.reshape(B, S, HID).astype(np.float32)


if __name__ == "__main__":
    rng = np.random.default_rng(0)
    inputs = {
        "query": rng.standard_normal((B, S, HID), dtype=np.float32),
        "key": rng.standard_normal((B, S, HID), dtype=np.float32),
        "value": rng.standard_normal((B, S, HID), dtype=np.float32),
        "attn_mask": rng.standard_normal((B, S, H, H), dtype=np.float32),
        "W_qkv": (rng.standard_normal((3 * HID, HID), dtype=np.float32)
                  / np.sqrt(HID)),
        "b_qkv": rng.standard_normal((3 * HID,), dtype=np.float32) * 0.01,
    }
    out = kernel(**inputs)
    print("kernel output:", out.shape, out.dtype, np.abs(out).mean())



# revision 3
# speedup vs baseline: 1.0128x; 1.0128x over previous
"""Trainium2 Bass kernel v3 for nn_Model1_52518860096440.

Reference (B=4, S=4096, HID=1024, H=16, DH=64):
    qkv = query @ W_qkv.T + b_qkv          # only `query` used
    q,k,v = split(qkv) -> (B,S,H,DH)
    s[t,h,g] = q[t,h]·k[t,g]/8 + mask[t,h,g]
    p = softmax_g(s);  o[t,h] = sum_g p[t,h,g] v[t,g]

v3 strategy (per core, 2048 tokens, 4 chunks of 512):
  - 3-term fp8e4 DoubleRow projection (4x PE rate vs f16 per term):
      qkv ~= xh8@Wh + Xl@Wh + xs16@Wl16
    with xh8=fp8(x), Xl=fp8(x-xh8), xs16=fp8(x/16), Wh=fp8(W),
    Wl16=fp8(16(W-Wh)).  All terms scale-1; accumulate in one PSUM group.
  - dense projection banks [128=(2 heads),512 tok]; ACT stage copy (f32->
    f16, q-side folds 1/8 score scale + bias); DVE/Pool aligned copies +
    4 batched shift DMAs/chunk build qAB/kAB tiles:
      qAB[64a+d, a*4096 + h*256 + u] = q[token(a,u), h, d], other half 0
    pairing: token(a,u) = chunk*512 + 256a + u.
  - scores TRANSPOSED per pair via one PE matmul (lhsT=k-block, rhs=
    q-block): s^T block [32(a,g), 32(a,h)] per pair; 64 pairs pack one
    PSUM bank [128,512] per 128-token tile. Off-diag quadrants exact 0.
  - softmax: DVE mask-add (mask host-prepped transposed w/ 0 off-diag),
    ACT exp(x-4), PE ones-matmul partition-group sums -> z[8,512], DVE
    recip * pat01, DMA partition-broadcast -> zb[128,512], DVE p=e*zb.
  - v projected in token layout, SBUF->SBUF scatter to V[(rho,a,g), 64v+d],
    DVE bias add; AV = 64 per-pair matmuls lhsT=p-block rhs=V-block ->
    o^T [32(a,h), 64d] blocks; ACT copy f16, DMA out; host unpermutes.
"""

from contextlib import ExitStack

import numpy as np

B, S, HID, H = 4, 4096, 1024, 16
DH = HID // H
NCORES = 8
T = B * S
TC = T // NCORES              # 2048 tokens/core
P = 128
CH = 512                      # tokens per chunk
NCH = TC // CH                # 4 chunks
NTS = 4                       # score tiles (128 tok) per chunk
NT = NCH * NTS                # 16 tiles/core
W3 = 3 * HID                  # 3072

_compiled = {}


def _cap(ap, dims, offset=None):
    """Copy `ap`, replace dims; `offset` is ADDED to the existing offset."""
    a = ap.copy()
    a.ap.clear()
    a.ap.extend([tuple(d) for d in dims])
    if offset is not None:
        a.offset = a.offset + offset
    return a


def _build(phase=4):
    import concourse.bass as bass
    import concourse.tile as tile
    import concourse.mybir as mybir
    from concourse import bacc

    f32 = mybir.dt.float32
    f16 = mybir.dt.float16
    f8 = mybir.dt.float8e4
    Act = mybir.ActivationFunctionType
    Alu = mybir.AluOpType
    DR = mybir.MatmulPerfMode.DoubleRow

    nc = bacc.Bacc("TRN2", target_bir_lowering=False, debug=False,
                   num_devices=NCORES)

    xh_d = nc.dram_tensor("xh", (HID, TC), f8, kind="ExternalInput")
    xl_d = nc.dram_tensor("xl", (HID, TC), f8, kind="ExternalInput")
    xs_d = nc.dram_tensor("xs", (HID, TC), f8, kind="ExternalInput")
    wh_d = nc.dram_tensor("wh", (HID, W3), f8, kind="ExternalInput")
    wl_d = nc.dram_tensor("wl", (HID, W3), f8, kind="ExternalInput")
    bqk_d = nc.dram_tensor("bqk", (P, 16), f32, kind="ExternalInput")
    bv_d = nc.dram_tensor("bvv", (P, DH), f16, kind="ExternalInput")
    maskT_d = nc.dram_tensor("maskT", (NT, P, 512), f16, kind="ExternalInput")
    pat_d = nc.dram_tensor("pat01", (8, 512), f16, kind="ExternalInput")
    ones8_d = nc.dram_tensor("ones8", (P, 8), f16, kind="ExternalInput")
    ones8T_d = nc.dram_tensor("ones8T", (8, P), f16, kind="ExternalInput")
    vstg_d = nc.dram_tensor("vstg", (NCH * 2, P, 2 * HID), f16,
                            kind="Internal")
    out_d = nc.dram_tensor("out", (NT, P, HID), f16, kind="ExternalOutput")
    if phase == 1:
        dbg_d = nc.dram_tensor("dbg", (NCH * 2, P, 2 * 4096), f16,
                               kind="ExternalOutput")
    if phase in (2, 3):
        dbg2_d = nc.dram_tensor("dbg2", (NT, P, 512), f16,
                                kind="ExternalOutput")

    with tile.TileContext(nc) as tc, ExitStack() as ctx:
        const = ctx.enter_context(tc.tile_pool(name="const", bufs=1))
        xp = ctx.enter_context(tc.tile_pool(name="xp", bufs=2))
        stp = ctx.enter_context(tc.tile_pool(name="stp", bufs=1))
        qkp = ctx.enter_context(tc.tile_pool(name="qkp", bufs=1))
        vsp = ctx.enter_context(tc.tile_pool(name="vsp", bufs=2))
        vvp = ctx.enter_context(tc.tile_pool(name="vvp", bufs=3))
        vbp = ctx.enter_context(tc.tile_pool(name="vbp", bufs=3))
        smp = ctx.enter_context(tc.tile_pool(name="smp", bufs=2))
        ep = ctx.enter_context(tc.tile_pool(name="ep", bufs=3))
        pp = ctx.enter_context(tc.tile_pool(name="pp", bufs=3))
        zzp = ctx.enter_context(tc.tile_pool(name="zzp", bufs=2))
        osp = ctx.enter_context(tc.tile_pool(name="osp", bufs=2))
        mkp = ctx.enter_context(tc.tile_pool(name="mkp", bufs=2))
        pq = ctx.enter_context(tc.tile_pool(name="pq", bufs=2, space="PSUM"))
        ps = ctx.enter_context(tc.tile_pool(name="ps", bufs=2, space="PSUM"))
        pz = ctx.enter_context(tc.tile_pool(name="pz", bufs=1, space="PSUM"))
        po = ctx.enter_context(tc.tile_pool(name="po", bufs=2, space="PSUM"))

        # ---------- resident constants ----------
        # w*_sb[p, kb*W3 + oc] = W^T[kb*128+p, oc]; loaded in 512-col slices
        # (interleaved wh/wl) so bank 0 can start after the first pair.
        wh_sb = const.tile([P, 8 * W3], f8, tag="wh")
        wl_sb = const.tile([P, 8 * W3], f8, tag="wl")

        def emit_w_loads(slices):
            for sl in slices:
                for wsb, wd in ((wh_sb, wh_d), (wl_sb, wl_d)):
                    nc.sync.dma_start(
                        _cap(wsb[:], [[8 * W3, P], [W3, 8], [1, 512]],
                             offset=sl * 512),
                        _cap(wd[:], [[W3, P], [P * W3, 8], [1, 512]],
                             offset=sl * 512))
        bqk_sb = const.tile([P, 16], f32, tag="bqk")
        bv_sb = const.tile([P, DH], f16, tag="bv")
        pat_sb = const.tile([P, 512], f16, tag="pat")
        ones8 = const.tile([P, 8], f16, tag="ones8")
        ones8T = const.tile([P, P], f16, tag="ones8T")
        neg4 = const.tile([P, 1], f32, tag="neg4")
        nc.vector.memset(neg4[:], -4.0)

        def emit_const_loads():
            nc.sync.dma_start(bv_sb[:], bv_d[:])
            nc.sync.dma_start(_cap(pat_sb[:], [[512, 8], [1, 512]]),
                              _cap(pat_d[:], [[512, 8], [1, 512]]))
            nc.sync.dma_start(ones8[:], ones8_d[:])
            nc.sync.dma_start(_cap(ones8T[:], [[P, 8], [1, P]]),
                              _cap(ones8T_d[:], [[P, 8], [1, P]]))

        # qAB/kAB tiles carry zeros in the unwritten partition halves;
        # memset each pool buffer once (writes never touch the zero regions
        # afterwards). Chunk-0's two buffers are zeroed immediately, one
        # piece per engine so neither blocks the first aligned copies; the
        # other two buffers are zeroed from inside round 0's work list.
        qk_bufs = [qkp.tile([P, 2 * 4096], f16, tag=f"qk{i}", name=f"qk{i}")
                   for i in range(4)]

        def emit_qk_zero(i, eng_a, eng_b):
            tl = qk_bufs[i]
            eng_a.memset(tl[64:128, 0:4096], 0.0)
            eng_b.memset(tl[0:64, 4096:8192], 0.0)

        emit_qk_zero(0, nc.vector, nc.gpsimd)
        emit_qk_zero(1, nc.gpsimd, nc.vector)

        state = {}
        attn_sts = {}
        prefetched = {}

        def emit_proj_bank(c, j, xts, stage):
            """Projection bank j (0-7 q, 8-15 k) of chunk c -> stage slice."""
            bank = pq.tile([P, CH], f32, tag="bank", name=f"bank{c}_{j}")
            n = 0
            for term in range(3):
                xt, wsb = xts[term]
                for kp in range(4):
                    lhsT = _cap(wsb[:], [[8 * W3, P], [W3, 2], [1, P]],
                                offset=(2 * kp) * W3 + j * P)
                    rhs = _cap(xt[:], [[8 * CH, P], [CH, 2], [1, CH]],
                               offset=(2 * kp) * CH)
                    nc.tensor.matmul(bank[:], lhsT, rhs,
                                     start=(n == 0), stop=(n == 11),
                                     perf_mode=DR)
                    n += 1
            scale = 0.125 if j < 8 else 1.0
            nc.scalar.activation(stage[:, j % 8 * 512:(j % 8 + 1) * 512],
                                 bank[:], Act.Identity,
                                 bias=bqk_sb[:, j:j + 1], scale=scale)

        def emit_aligned(c, j, stage, qab, eng):
            jj = j % 8
            # h=2j A-half (aligned at partitions 0:64) -> qA
            eng.tensor_copy(
                qab[0:64, (2 * jj) * 256:(2 * jj) * 256 + 256],
                stage[0:64, jj * 512:jj * 512 + 256])
            # h=2j+1 B-half (aligned at partitions 64:128) -> qB
            eng.tensor_copy(
                qab[64:128, 4096 + (2 * jj + 1) * 256:
                    4096 + (2 * jj + 1) * 256 + 256],
                stage[64:128, jj * 512 + 256:jj * 512 + 512])

        def emit_shifts(stage, qab, jlo=0, jn=8):
            # h-odd A-half: stage[64:128, j*512+0:+256] -> qA[0:64, (2j+1)*256]
            nc.sync.dma_start(
                _cap(qab[:], [[8192, 64], [512, jn], [1, 256]],
                     offset=256 + jlo * 512),
                _cap(stage[:], [[4096, 64], [512, jn], [1, 256]],
                     offset=64 * 4096 + jlo * 512))
            # h-even B-half: stage[0:64, j*512+256:+256] -> qB[64:128, 2j*256]
            nc.sync.dma_start(
                _cap(qab[:], [[8192, 64], [512, jn], [1, 256]],
                     offset=64 * 8192 + 4096 + jlo * 512),
                _cap(stage[:], [[4096, 64], [512, jn], [1, 256]],
                     offset=256 + jlo * 512))

        def emit_vproj(c, vb2, xts_l, vtiles):
            """v half-bank vb2 = (tb, half): token block tb, chan half."""
            tb, half = vb2
            bank = pq.tile([P, CH], f32, tag="bank", name=f"vb{c}_{tb}_{half}")
            n = 0
            for term in range(3):
                xt, wsb = xts_l[term]
                for kp in range(4):
                    lhsT = _cap(xt[:], [[8 * CH, P], [CH, 2], [1, P]],
                                offset=(2 * kp) * CH + tb * P)
                    rhs = _cap(wsb[:], [[8 * W3, P], [W3, 2], [1, 512]],
                               offset=(2 * kp) * W3 + 2 * HID + half * 512)
                    nc.tensor.matmul(bank[:], lhsT, rhs,
                                     start=(n == 0), stop=(n == 11),
                                     perf_mode=DR)
                    n += 1
            nc.scalar.copy(
                vtiles[tb % 2][:, (tb // 2) * HID + half * 512:
                               (tb // 2) * HID + (half + 1) * 512],
                bank[:])

        # ---- attention stages for chunk ca, tile tau ----
        def emit_A2(ca, tau, st):
            # V-gather from DRAM: V[32r+16a+g, 64v+d] = v[token(r,v,a), g, d]
            V = vvp.tile([P, HID], f16, tag="V")
            vsrc = vstg_d[ca * 2 + (tau >> 1)]
            iengs = [nc.sync, nc.gpsimd, nc.scalar, nc.gpsimd]
            for r in range(4):
                iengs[r].dma_start(
                    _cap(V[:], [[HID, 32], [DH, 16], [1, DH]],
                         offset=32 * r * HID),
                    _cap(vsrc[:], [[DH, 32], [2 * HID, 16], [1, DH]],
                         offset=(64 * (tau & 1) + 16 * r) * 2 * HID))
            vb = vbp.tile([P, HID], f16, tag="vb")
            nc.vector.tensor_add(
                vb[:].rearrange("p (v d) -> p v d", v=16),
                V[:].rearrange("p (v d) -> p v d", v=16),
                bv_sb[:].unsqueeze(1).broadcast_to((P, 16, DH)))
            st.update(vb=vb)

        def emit_A1(ca, tau, st):
            qab, kab, mk = (state[ca][k] for k in ("qab", "kab", "mk"))
            # scores
            sps = ps.tile([P, 512], f32, tag="sps", name=f"s{ca}_{tau}")
            for w in range(64):
                r, v = w // 16, w % 16
                u = 64 * tau + w
                lhsT = _cap(kab[:], [[2 * 4096, P], [4096, 2], [256, 16]],
                            offset=u)
                rhs = _cap(qab[:], [[2 * 4096, P], [4096, 2], [256, 16]],
                           offset=u)
                nc.tensor.matmul(sps[32 * r:32 * r + 32, 32 * v:32 * v + 32],
                                 lhsT, rhs, start=True, stop=True,
                                 tile_position=(0, 32 * r))
            sm = smp.tile([P, 512], f16, tag="sm")
            nc.vector.tensor_add(sm[:], sps[:],
                                 mk[:, tau * 512:(tau + 1) * 512])
            if phase == 2:
                nc.sync.dma_start(dbg2_d[ca * NTS + tau], sm[:])
                st.update(done=True)
                return
            e = ep.tile([P, 512], f16, tag="e")
            nc.scalar.activation(e[:], sm[:], Act.Exp, bias=neg4[:])
            st.update(e=e)

        def emit_B(ca, tau, st):
            if st.get("done"):
                return
            e = st["e"]
            zps = pz.tile([P, 512], f32, tag="z", name=f"z{ca}_{tau}")
            nc.tensor.matmul(zps[0:8, :], ones8[:], e[:],
                             start=True, stop=True)
            rz = zzp.tile([P, 512], f16, tag="rz")
            with nc.allow_low_precision(reason="softmax recip f16 ok"):
                nc.vector.reciprocal(rz[0:8, :], zps[0:8, :])
            rz2 = zzp.tile([P, 512], f16, tag="rz2")
            nc.vector.tensor_mul(rz2[0:8, :], rz[0:8, :], pat_sb[0:8, :])
            zb = pz.tile([P, 512], f32, tag="zb", name=f"zb{ca}_{tau}")
            nc.tensor.matmul(zb[:], ones8T[0:8, :], rz2[0:8, :],
                             start=True, stop=True)
            p_t = pp.tile([P, 512], f16, tag="p")
            nc.vector.tensor_mul(p_t[:], e[:], zb[:])
            if phase == 3:
                nc.sync.dma_start(dbg2_d[ca * NTS + tau], p_t[:])
                st.update(done=True)
                return
            st.update(p=p_t)

        def emit_A(ca, tau, st):
            emit_A2(ca, tau, st)
            emit_A1(ca, tau, st)

        def emit_C(ca, tau, st):
            if st.get("done"):
                return
            p_t, vb = st["p"], st["vb"]
            osb = osp.tile([P, HID], f16, tag="osb")
            # o^T blocks [64d, 32(a,h)]; pe tied to row-tile (walrus chokes on
            # pe alternating across row-position changes)
            banks = [po.tile([P, 512], f32, tag="ops",
                             name=f"o{ca}_{tau}_{b}") for b in range(2)]
            for r in range(4):
                pe = 64 * (r & 1)
                for v in range(16):
                    col = 256 * (r >> 1) + 32 * (v % 8)
                    nc.tensor.matmul(
                        banks[v // 8][pe:pe + 64, col:col + 32],
                        vb[32 * r:32 * r + 32, 64 * v:64 * v + 64],
                        p_t[32 * r:32 * r + 32, 32 * v:32 * v + 32],
                        start=True, stop=True,
                        tile_position=(32 * r, pe))
            if ca == NCH - 1 and tau == NTS - 1:
                # final tile: pipeline the epilogue (parallel copies, eager
                # half stores) to shorten the terminal latency chain
                nc.scalar.copy(osb[:, 0:512], banks[0][:])
                nc.sync.dma_start(
                    _cap(out_d[ca * NTS + tau], [[HID, P], [1, 512]]),
                    osb[:, 0:512])
                nc.vector.tensor_copy(osb[:, 512:1024], banks[1][:])
                nc.scalar.dma_start(
                    _cap(out_d[ca * NTS + tau], [[HID, P], [1, 512]],
                         offset=512),
                    osb[:, 512:1024])
            else:
                for b in range(2):
                    nc.scalar.copy(osb[:, b * 512:(b + 1) * 512], banks[b][:])
                nc.scalar.dma_start(out_d[ca * NTS + tau], osb[:])

        # ---------- interleaved emission ----------
        def emit_xmask(c):
            xh = xp.tile([P, 8 * CH], f8, tag="xh", name=f"xh{c}")
            xl = xp.tile([P, 8 * CH], f8, tag="xl", name=f"xl{c}")
            xs = xp.tile([P, 8 * CH], f8, tag="xs", name=f"xs{c}")
            for i, (xt, xd) in enumerate(
                    ((xh, xh_d), (xl, xl_d), (xs, xs_d))):
                nc.sync.dma_start(
                    _cap(xt[:], [[8 * CH, P], [CH, 8], [1, CH]]),
                    _cap(xd[:], [[TC, P], [P * TC, 8], [1, CH]],
                         offset=c * CH))
                if c == 0 and i == 0:
                    emit_w_loads([0])
            mk = mkp.tile([P, NTS * 512], f16, tag="mk", name=f"mk{c}")
            nc.sync.dma_start(
                _cap(mk[:], [[NTS * 512, P], [512, NTS], [1, 512]]),
                _cap(maskT_d[:], [[512, P], [P * 512, NTS], [1, 512]],
                     offset=c * NTS * P * 512))
            return xh, xl, xs, mk

        def emit_round(cp, ca):
            if cp is not None:
                if cp == 0:
                    xh, xl, xs, mk = emit_xmask(0)
                    nc.sync.dma_start(bqk_sb[:], bqk_d[:])
                    emit_w_loads([1])
                    emit_const_loads()
                    emit_w_loads(range(2, 6))
                else:
                    xh, xl, xs, mk = emit_xmask(cp)
                xts = [(xh, wh_sb), (xl, wh_sb), (xs, wl_sb)]
                stage_q = stp.tile([P, 4096], f16, tag="stq")
                stage_k = stp.tile([P, 4096], f16, tag="stk")
                qab = qk_bufs[(cp % 2) * 2]
                kab = qk_bufs[(cp % 2) * 2 + 1]
                vtiles = [vsp.tile([P, 2 * HID], f16, tag=f"vt{i}",
                                   name=f"vt{cp}_{i}")
                          for i in range(2)]

                def mk_bank(j):
                    def f():
                        stage = stage_q if j < 8 else stage_k
                        ab = qab if j < 8 else kab
                        emit_proj_bank(cp, j, xts, stage)
                        eng = nc.vector if j % 2 == 0 else nc.gpsimd
                        emit_aligned(cp, j, stage, ab, eng)
                        if cp == 0 and j in (2, 4):
                            emit_qk_zero(2 + (j == 4),
                                         nc.vector if j == 4 else nc.gpsimd,
                                         nc.gpsimd if j == 4 else nc.vector)
                    return f

                def mk_shift(stage, ab, dump_i, jlo=0, jn=8):
                    def f():
                        emit_shifts(stage, ab, jlo, jn)
                        if phase == 1 and jlo + jn == 8:
                            nc.sync.dma_start(dbg_d[cp * 2 + dump_i], ab[:])
                    return f

                def mk_vproj(tb, half):
                    return lambda: emit_vproj(cp, (tb, half), xts, vtiles)

                pw = [mk_bank(j) for j in range(16)]
                pw.insert(12, mk_shift(stage_k, kab, 1, 0, 4))
                pw.insert(8, mk_shift(stage_q, qab, 0))
                pw.append(mk_shift(stage_k, kab, 1, 4, 4))
                pw += [mk_vproj(tb, half) for tb in range(4)
                       for half in range(2)]

                def mk_vstg(tt):
                    return lambda: nc.sync.dma_start(
                        vstg_d[cp * 2 + tt], vtiles[tt][:])
                pw += [mk_vstg(0), mk_vstg(1)]
                state[cp] = dict(qab=qab, kab=kab, mk=mk, v=vtiles)
            else:
                pw = []

            aw = []
            fns = {0: emit_A, 1: emit_B, 2: emit_C}
            if ca is not None and phase != 1:
                sts = attn_sts.setdefault(ca, [dict() for _ in range(NTS)])
                if cp is None:
                    aw = []
                else:
                    for tau in range(NTS):
                        aw.append((0, tau))
                        if tau >= 1:
                            aw.append((1, tau - 1))
                        if tau >= 2:
                            aw.append((2, tau - 2))
                    aw += [(1, NTS - 1), (2, NTS - 2), (2, NTS - 1)]
                aw = [(lambda s=stage, t=tau: fns[s](ca, t, sts[t]))
                      for stage, tau in aw]
            own = []
            if cp == NCH - 1 and phase != 1:
                # last chunk: v-proj + vstg go FIRST so V-gathers (A2) can
                # run during the qk projection; A1 after the shifts; B/C
                # at the end of the round.
                sts_own = attn_sts.setdefault(cp, [dict()
                                                   for _ in range(NTS)])
                qk_part, v_part = pw[:19], pw[19:]
                a2 = [(lambda t=tau: emit_A2(cp, t, sts_own[t]))
                      for tau in range(NTS)]
                pw = (v_part + a2[:2] + qk_part[:9] + a2[2:]
                      + qk_part[9:])
                a1 = [(lambda t=tau: emit_A1(cp, t, sts_own[t]))
                      for tau in range(NTS)]
                seq = [(1, 0), (1, 1), (2, 0), (1, 2), (2, 1),
                       (1, 3), (2, 2), (2, 3)]
                fns2 = {1: emit_B, 2: emit_C}
                own = a1 + [(lambda s=stage, t=tau:
                             fns2[s](cp, t, sts_own[t]))
                            for stage, tau in seq]

            # interleave: spread attention work between projection banks
            if pw and aw:
                out_sched = []
                na, npw = len(aw), len(pw)
                ai = 0
                for i, w in enumerate(pw):
                    out_sched.append(w)
                    want = (i + 1) * na // npw
                    while ai < want:
                        out_sched.append(aw[ai])
                        ai += 1
                out_sched += aw[ai:]
            else:
                out_sched = pw + aw
            for w in out_sched:
                w()
            for w in own:
                w()

        for r in range(NCH + 1):
            emit_round(r if r < NCH else None,
                       r - 1 if r >= 1 else None)

    nc.compile()
    return nc


def _host_prep(query, W_qkv, b_qkv, attn_mask):
    import ml_dtypes
    F8 = ml_dtypes.float8_e4m3

    def fp8(a):
        return np.asarray(a, dtype=F8)

    x = np.ascontiguousarray(query.reshape(T, HID), dtype=np.float32)
    xT = np.ascontiguousarray(x.T)                     # (HID, T)
    xh8 = fp8(xT)
    xl8 = fp8(xT - xh8.astype(np.float32))
    xs8 = fp8(xT / 16.0)

    Wt = np.array(W_qkv, dtype=np.float32).T.copy()    # (HID, 3H)
    wh8 = fp8(Wt)
    wl8 = fp8(16.0 * (Wt - wh8.astype(np.float32)))

    b = np.array(b_qkv, dtype=np.float32)
    bqk = np.empty((P, 16), dtype=np.float32)
    for j in range(8):
        bqk[:, j] = b[j * P:(j + 1) * P] / 8.0
        bqk[:, 8 + j] = b[HID + j * P:HID + (j + 1) * P]
    bvv = np.ascontiguousarray(
        np.tile(b[2 * HID:].reshape(H, DH), (8, 1))).astype(np.float16)

    # maskT[gt, 32r+16a+g, 32v+16a+h] = mask[t(gt,r,v,a), h, g]
    m = np.asarray(attn_mask, dtype=np.float32).reshape(T, H, H)
    gt = np.arange(NT)[:, None, None, None]
    rr = np.arange(4)[None, :, None, None]
    aa = np.arange(2)[None, None, :, None]
    vv = np.arange(16)[None, None, None, :]
    # token within core: c*512 + 256a + 64tau + 4v + r
    tok = ((gt // NTS) * CH + 256 * aa + 64 * (gt % NTS) + 16 * rr + vv)
    # dims [gt, r, a, g, v, a', h]; off-diag (a != a') stays zero
    maskT = np.zeros((NCORES, NT, 4, 2, 16, 16, 2, 16), dtype=np.float16)
    for core in range(NCORES):
        mc = m[core * TC:(core + 1) * TC]             # (TC, H, H)
        blk = mc[tok]                                  # [gt, r, a, v, h, g]
        for a in range(2):
            # dst [gt, r, g, v, h] = blk[gt, r, a, v, h, g]
            maskT[core, :, :, a, :, :, a, :] = \
                np.moveaxis(blk[:, :, a], [2, 3, 4], [3, 4, 2])
    maskT = maskT.reshape(NCORES, NT, P, 512)

    pat01 = np.zeros((8, 2, 16), dtype=np.float16)
    for jj in range(8):
        pat01[jj, jj % 2, :] = 1.0
    pat01 = np.tile(pat01.reshape(8, 32), (1, 16))     # (8, 512)

    ones8 = np.zeros((P, 8), dtype=np.float16)
    for jj in range(8):
        ones8[16 * jj:16 * jj + 16, jj] = 1.0
    ones8T = np.ascontiguousarray(ones8.T)

    return xh8, xl8, xs8, wh8, wl8, bqk, bvv, maskT, pat01, ones8, ones8T


def _unpermute(res):
    # res: (NT, 128, 1024) f16, o^T blocks at
    # [64*(r&1)+d, 512*(v//8) + 256*(r>>1) + 32*(v%8) + 16a + h]
    r = res.astype(np.float32).reshape(NCH, NTS, 2, DH, 2, 2, 8, 2, 16)
    # dims [c, tau, e, d, bank, r2, v8, a, h]; r = 2*r2+e, v = 8*bank+v8
    # token = c*512 + 256a + 64tau + 16r + v -> (c, a, tau, r2, e, bank, v8)
    r = r.transpose(0, 7, 1, 5, 2, 4, 6, 8, 3)
    return np.ascontiguousarray(r).reshape(TC, HID)


def kernel(query, key, value, attn_mask, W_qkv, b_qkv):
    from concourse.bass_utils import run_bass_kernel_spmd

    (xh8, xl8, xs8, wh8, wl8, bqk, bvv, maskT, pat01,
     ones8, ones8T) = _host_prep(query, W_qkv, b_qkv, attn_mask)

    if "nc" not in _compiled:
        _compiled["nc"] = _build()
    nc = _compiled["nc"]

    in_maps = []
    for c in range(NCORES):
        tsl = slice(c * TC, (c + 1) * TC)
        in_maps.append({
            "xh": np.ascontiguousarray(xh8[:, tsl]),
            "xl": np.ascontiguousarray(xl8[:, tsl]),
            "xs": np.ascontiguousarray(xs8[:, tsl]),
            "wh": wh8,
            "wl": wl8,
            "bqk": bqk,
            "bvv": bvv,
            "maskT": maskT[c],
            "pat01": pat01,
            "ones8": ones8,
            "ones8T": ones8T,
        })

    res = run_bass_kernel_spmd(nc, in_maps, core_ids=list(range(NCORES)))
    out = np.concatenate([_unpermute(r["out"]) for r in res.results], axis=0)
    return out.reshape(B, S, HID).astype(np.float32)


if __name__ == "__main__":
    rng = np.random.default_rng(0)
    inputs = {
        "query": rng.standard_normal((B, S, HID), dtype=np.float32),
        "key": rng.standard_normal((B, S, HID), dtype=np.float32),
        "value": rng.standard_normal((B, S, HID), dtype=np.float32),
        "attn_mask": rng.standard_normal((B, S, H, H), dtype=np.float32),
        "W_qkv": (rng.standard_normal((3 * HID, HID), dtype=np.float32)
                  / np.sqrt(HID)),
        "b_qkv": rng.standard_normal((3 * HID,), dtype=np.float32) * 0.01,
    }
    out = kernel(**inputs)
    print("kernel output:", out.shape, out.dtype, np.abs(out).mean())
